# revision 1
# baseline (speedup 1.0000x reference)
"""BiLSTM layer kernel for 8 Trainium2 NeuronCores — v2.

Problem: B=32, T=256, D=1024, H=1024 bidirectional LSTM.
Sharding: data-parallel over batch (4 rows/core) x 2 directions = 8
sequences per core; weights replicated; time recurrence local per core.

v2 changes vs baseline:
  - Elementwise ops cover all 4 bands in single 128-partition
    instructions (bands live at partition offsets 0/32/64/96; the 24
    garbage rows per band are zero-filled by 32-wide preadd matmuls).
  - Gate-major streaming order (f+g fused 512 | i | o_lo | o_hi) with
    the o gate last: the post-stream critical chain is only
    sigmoid(o) -> h-mul -> transpose -> copy.  f/g/i tanh and the
    cell-state update all overlap the tail of the weight stream.
  - One PSUM bank per chunk (gp_A, gp_B double-buffered; gp_Cl/gp_Ch
    single): bank-granularity PSUM deps never serialize the chain,
    and sigm_lo doesn't wait on the o_hi matmuls.  Exactly one
    start=True matmul per (partition range x bank) — a second one
    re-arms bank zeroing and wipes the first's columns.
  - h transposed back to stationary layout with two full 128x128 PE
    transposes (+2 copies) instead of 8 narrow transposes + 4 copies;
    even-k matmuls of the next step only wait on the lo-half copy.
  - h, gate activations, ring buffers in bf16 (DVE 2x mode; half DMA).
  - Both directions' outputs ride 8-deep rings flushed as batched DMAs
    (gpsimd copies h into the reversed-slot bwd ring off the critical
    path).  Per-step DMAs issue from the SP queue: gpsimd-issued DMAs
    cost ~1us of real engine time each (SWDGE descriptor generation).
  - Phase 1 n-outer so only one 512-col strip of Wx is SBUF-resident;
    per-k-tile chunked loads ordered to unblock the first matmuls
    early; Wh's load is spread across the n-loop.

Math (identical to baseline): sigmoid via tanh for f,i — the x/2
scaling is baked into the host-prepped weights; S = 2*c is carried so
S' = ((tanh(f/2)+1)*S)*0.5 + (tanh(i/2)+1)*tanh(g) and
h = sigmoid(o) * tanh(S'/2).
"""

import numpy as np

B, T, D, H = 32, 256, 1024, 1024
NCORES = 8
BS = B // NCORES           # batch rows per core
KT = D // 128              # k-tiles over the hidden/contraction dim
G4 = 4 * H                 # gate columns
RING = 8                   # fwd output ring depth (steps per output DMA)

_BUILT = None


def _build(nc_T=T):
    import concourse.bass as bass
    import concourse.bacc as bacc
    import concourse.tile as tile
    from concourse import mybir

    f32 = mybir.dt.float32
    f32r = mybir.dt.float32r
    bf16 = mybir.dt.bfloat16

    nc = bacc.Bacc("TRN2", target_bir_lowering=False)

    nrows = BS * nc_T
    xT = nc.dram_tensor("xT", [D + 1, nrows], f32r, kind="ExternalInput")
    wxb = nc.dram_tensor("wxb", [D + 1, G4], f32r, kind="ExternalInput")
    wh = nc.dram_tensor("wh", [D, G4], bf16, kind="ExternalInput")
    sel = nc.dram_tensor("sel", [128, 128], bf16, kind="ExternalInput")
    biasz = nc.dram_tensor("biasz", [64, 1024], bf16, kind="ExternalInput")
    id128 = nc.dram_tensor("id128", [128, 128], bf16, kind="ExternalInput")
    outd = nc.dram_tensor("out", [nc_T, 2, 4, BS, 256], bf16,
                          kind="ExternalOutput")
    pred = nc.dram_tensor("pre", [nc_T, 2, 16, 1024], bf16, kind="Internal")
    import os
    dbg = None
    if os.environ.get("K_DEBUG"):
        dbg = (nc.dram_tensor("dbg_hT", [2, 128, 128], f32,
                              kind="ExternalOutput"),
               nc.dram_tensor("dbg_gp", [128, 1024], f32,
                              kind="ExternalOutput"),
               nc.dram_tensor("dbg_gp2", [128, 1024], f32,
                              kind="ExternalOutput"))

    with tile.TileContext(nc) as tc:
        _emit(nc, tc, bass, mybir, nc_T, xT, wxb, wh, sel, biasz, id128,
              outd, pred, dbg)
    nc.finalize()
    return nc


def _emit(nc, tc, bass, mybir, nc_T, xT, wxb, wh, sel, biasz, id128, outd,
          pred, dbg=None):
    from contextlib import ExitStack

    f32 = mybir.dt.float32
    f32r = mybir.dt.float32r
    bf16 = mybir.dt.bfloat16
    Tanh = mybir.ActivationFunctionType.Tanh
    Sigm = mybir.ActivationFunctionType.Sigmoid
    Copy = mybir.ActivationFunctionType.Copy
    MUL = mybir.AluOpType.mult
    ADD = mybir.AluOpType.add

    nrows = BS * nc_T
    act = nc.scalar
    dve = nc.vector
    pool = nc.gpsimd
    pe = nc.tensor
    sdma = nc.sync

    with ExitStack() as ctx:
        # ---------------- constants + big persistent weights ----------------
        singles = ctx.enter_context(tc.tile_pool(name="singles", bufs=1))
        sel_sb = singles.tile([128, 128], bf16)
        id_sb = singles.tile([128, 128], bf16)
        sdma.dma_start(out=sel_sb, in_=sel[:, :])
        sdma.dma_start(out=id_sb, in_=id128[:, :])

        whpool = ctx.enter_context(tc.tile_pool(name="whp", bufs=1))
        wh_sb = whpool.tile([128, KT, G4], bf16)

        # ---------------- phase 1: pre-gates (x @ Wx + b) ----------------
        with tc.tile_pool(name="p1x", bufs=1) as p1x, \
             tc.tile_pool(name="p1w", bufs=2) as p1w, \
             tc.tile_pool(name="p1o", bufs=3) as p1o, \
             tc.tile_pool(name="p1ps", bufs=3, space="PSUM") as p1ps:
            xT_sb = p1x.tile([128, KT, nrows], f32r)

            n_mtiles = (nrows + 127) // 128
            wxbn = [None] * 8

            def load_wxbn(n):
                wxbn[n] = p1w.tile([128, KT, 512], f32r, tag="wxbn",
                                   name=f"wxbn{n}")
                for k in range(KT):
                    sdma.dma_start(
                        out=wxbn[n][:, k, :],
                        in_=wxb[k * 128:(k + 1) * 128,
                                n * 512:(n + 1) * 512])

            # interleave the first wxbn strip with xT so the first
            # m-tile's operands land earliest
            wxbn[0] = p1w.tile([128, KT, 512], f32r, tag="wxbn",
                               name="wxbn0")
            for k in range(KT):
                sdma.dma_start(
                    out=wxbn[0][:, k, :],
                    in_=wxb[k * 128:(k + 1) * 128, 0:512])
                sdma.dma_start(
                    out=xT_sb[:, k, :],
                    in_=xT[k * 128:(k + 1) * 128, :])
            load_wxbn(1)
            for n in range(8):
                if n + 2 < 8:
                    load_wxbn(n + 2)
                # spread the Wh load (needed only by phase 2) across the
                # n-loop so it never starves the phase-1 input DMAs
                sdma.dma_start(
                    out=wh_sb[:, n, :],
                    in_=wh[n * 128:(n + 1) * 128, :])
                g = n // 2
                half = n % 2
                for m in range(n_mtiles):
                    mrows = min(128, nrows - m * 128)
                    pt = p1ps.tile([128, 512], f32, name="pt")
                    for k in range(KT):
                        pe.matmul(
                            pt[0:mrows, :],
                            lhsT=xT_sb[:, k, m * 128:m * 128 + mrows],
                            rhs=wxbn[n][:, k, :],
                            start=(k == 0), stop=(k == KT - 1))
                    ot = p1o.tile([128, 512], bf16, name="ot")
                    ol = p1o.tile([128, 512], bf16, name="ol")
                    dve.tensor_copy(ot[0:mrows, :], pt[0:mrows, :])
                    dve.tensor_sub(ol[0:mrows, :], pt[0:mrows, :],
                                   ot[0:mrows, :])
                    # rows r = m*128+p -> (s = r // nc_T, t = r % nc_T)
                    p0 = 0
                    while p0 < mrows:
                        s = (m * 128 + p0) // nc_T
                        t0 = (m * 128 + p0) % nc_T
                        cnt = min(mrows - p0, nc_T - t0)
                        act.dma_start(
                            out=pred[t0:t0 + cnt, 0, 4 * g + s,
                                     half * 512:(half + 1) * 512],
                            in_=ot[p0:p0 + cnt, :])
                        act.dma_start(
                            out=pred[t0:t0 + cnt, 1, 4 * g + s,
                                     half * 512:(half + 1) * 512],
                            in_=ol[p0:p0 + cnt, :])
                        p0 += cnt

        # ---------------- phase 2: the recurrence ----------------
        state = ctx.enter_context(tc.tile_pool(name="state", bufs=1))
        prepool = ctx.enter_context(tc.tile_pool(name="pre", bufs=3))
        ewpool = ctx.enter_context(tc.tile_pool(name="ew", bufs=2))
        ringpool = ctx.enter_context(tc.tile_pool(name="ring", bufs=1))
        gpool = ctx.enter_context(tc.tile_pool(name="gp", bufs=2,
                                               space="PSUM"))
        gpoolo = ctx.enter_context(tc.tile_pool(name="gpo", bufs=1,
                                                space="PSUM"))
        tpool = ctx.enter_context(tc.tile_pool(name="tp", bufs=1,
                                               space="PSUM"))

        # persistent state
        S_sb = state.tile([128, 256], f32, tag="S")          # 2*c, band rows
        hT0 = state.tile([128, 128], bf16, tag="hT0")        # dims kk=0, col 32b+s
        hT1 = state.tile([128, 128], bf16, tag="hT1")        # dims kk=1
        ring_lo = ringpool.tile([128, RING, 128], bf16, tag="rlo")
        ring_hi = ringpool.tile([128, RING, 128], bf16, tag="rhi")
        ring_blo = ringpool.tile([128, RING, 128], bf16, tag="rblo")
        ring_bhi = ringpool.tile([128, RING, 128], bf16, tag="rbhi")
        dve.memset(S_sb, 0.0)

        # gate quarter -> band-local column range
        QF, QG, QI, QO = 0, 1, 2, 3     # stream order f, g, i, o

        prev = None  # (ring_lo slot APs, ...) for deferred transpose

        for tau in range(nc_T):
            slot = tau % RING

            # ---- prefetch pre-gate rows two steps ahead (gpsimd queue) ----
            pf = tau + 2
            if tau == 0:
                for pf0 in (0, 1):
                    pfb = prepool.tile([128, 1024], bf16, tag="pfb",
                                       name=f"pfb{pf0}")
                    sdma.dma_start(out=pfb[0:32, :], in_=pred[pf0, :, :, :])
                    sdma.dma_start(out=pfb[32:64, :],
                                   in_=pred[nc_T - 1 - pf0, :, :, :])
                    sdma.dma_start(out=pfb[64:128, :], in_=biasz[:, :])
                    if pf0 == 0:
                        pfb_q = [pfb]
                    else:
                        pfb_q.append(pfb)
            if pf < nc_T:
                pfb = prepool.tile([128, 1024], bf16, tag="pfb",
                                   name=f"pfb{pf}")
                sdma.dma_start(out=pfb[0:32, :], in_=pred[pf, :, :, :])
                sdma.dma_start(out=pfb[32:64, :],
                               in_=pred[nc_T - 1 - pf, :, :, :])
                sdma.dma_start(out=pfb[64:128, :], in_=biasz[:, :])
                pfb_q.append(pfb)
            pfb = pfb_q[tau]

            # ---- PSUM gate tiles for this step ----
            # one bank per chunk; o is split across two banks so sigm_lo
            # does not wait on the o_hi matmuls (PSUM deps are per bank)
            gp_A = gpool.tile([128, 512], f32, tag="gpA", name="gpA")  # f|g
            gp_B = gpool.tile([128, 256], f32, tag="gpB", name="gpB")  # i
            gp_Cl = gpoolo.tile([128, 128], f32, tag="gpCl", name="gpCl")
            gp_Ch = gpoolo.tile([128, 128], f32, tag="gpCh", name="gpCh")

            # ---- pre-gate injection (PE, K=64, runs during prev tail) ----
            # 32-wide stationary (24 zero cols) so the whole 128-partition
            # PSUM tile is written: the elementwise phase reads all rows.
            # ONE start=True matmul per (partition range x PSUM bank):
            # start=True re-arms zeroing for the whole bank region, so a
            # second start=True in the same bank would wipe the first
            # preadd's columns on the next accumulating matmul.
            first = (tau == 0)
            for d, c0, c1 in ((gp_A, 0, 512), (gp_B, 512, 768),
                              (gp_Cl, 768, 896), (gp_Ch, 896, 1024)):
                for b in range(4):
                    pe.matmul(
                        d[32 * b:32 * b + 32, :],
                        lhsT=sel_sb[:, 32 * b:32 * b + 32],
                        rhs=pfb[:, c0:c1],
                        start=True, stop=first,
                        tile_position=(0, 32 * b),
                        skip_group_check=True)

            # ---- deferred: transpose h(tau-1) into stationary layout ----
            if prev is not None:
                p_lo, p_hi = prev
                pe.transpose(tp0[:, :], in_=p_lo, identity=id_sb)
                pe.transpose(tp1[:, :], in_=p_hi, identity=id_sb)
                dve.tensor_copy(hT0, tp0[:, :])
                act.activation(hT1, tp1[:, :], Copy)
                if dbg is not None and tau == 1:
                    dhT = ewpool.tile([128, 2, 128], f32, tag="dbght",
                                      name="dhT")
                    dve.tensor_copy(dhT[:, 0, :], hT0)
                    dve.tensor_copy(dhT[:, 1, :], hT1)
                    pool.dma_start(out=dbg[0][0, :, :], in_=dhT[:, 0, :])
                    pool.dma_start(out=dbg[0][1, :, :], in_=dhT[:, 1, :])

            if dbg is not None and tau == 1:
                dgp2 = ewpool.tile([128, 1024], f32, tag="dbggp2",
                                   name="dgp2")
                dve.tensor_copy(dgp2[:, 0:512], gp_A)
                dve.tensor_copy(dgp2[:, 512:768], gp_B)
                dve.tensor_copy(dgp2[:, 768:896], gp_Cl)
                dve.tensor_copy(dgp2[:, 896:1024], gp_Ch)
                pool.dma_start(out=dbg[2][:, :], in_=dgp2)

            # ---- the weight stream (PE): quarters f, g, i, o_lo, o_hi ----
            if not first:
                korder = (0, 2, 4, 6, 1, 3, 5, 7)
                # (dst tile, dst col range, band-local col range)
                chunks = ((gp_A, 0, 512, 0, 512),      # f+g fused
                          (gp_B, 0, 256, 512, 768),    # i
                          (gp_Cl, 0, 128, 768, 896),   # o lo
                          (gp_Ch, 0, 128, 896, 1024))  # o hi
                for d, d0, d1, c0, c1 in chunks:
                    for ki, k in enumerate(korder):
                        hT = (hT0, hT1)[k % 2]
                        kb = k // 2
                        last = (ki == len(korder) - 1)
                        for b in range(4):
                            pe.matmul(
                                d[32 * b:32 * b + 8, d0:d1],
                                lhsT=hT[:, 32 * kb:32 * kb + 8],
                                rhs=wh_sb[:, k,
                                          b * 1024 + c0:b * 1024 + c1],
                                start=False, stop=last,
                                tile_position=(0, 32 * b),
                                skip_group_check=True)

            if dbg is not None and tau == 1:
                dgp = ewpool.tile([128, 1024], f32, tag="dbggp", name="dgp")
                dve.tensor_copy(dgp[:, 0:512], gp_A)
                dve.tensor_copy(dgp[:, 512:768], gp_B)
                dve.tensor_copy(dgp[:, 768:896], gp_Cl)
                dve.tensor_copy(dgp[:, 896:1024], gp_Ch)
                pool.dma_start(out=dbg[1][:, :], in_=dgp)

            # ---- elementwise cell update ----
            tfg = ewpool.tile([128, 512], bf16, tag="tfg", name="tfg")
            ti = ewpool.tile([128, 256], bf16, tag="ti", name="ti")
            so_lo = ewpool.tile([128, 128], bf16, tag="solo", name="so_lo")
            so_hi = ewpool.tile([128, 128], bf16, tag="sohi", name="so_hi")
            a_t = ewpool.tile([128, 256], f32, tag="a", name="a_t")
            b_t = ewpool.tile([128, 256], f32, tag="b", name="b_t")
            tcS = ewpool.tile([128, 256], bf16, tag="tcS", name="tcS")

            act.activation(tfg, gp_A[:, 0:512], Tanh)
            dve.scalar_tensor_tensor(a_t, tfg[:, 0:256], 1.0, S_sb,
                                     op0=ADD, op1=MUL)
            act.activation(ti, gp_B[:, :], Tanh)
            dve.scalar_tensor_tensor(b_t, ti, 1.0, tfg[:, 256:512],
                                     op0=ADD, op1=MUL)
            dve.scalar_tensor_tensor(S_sb, a_t, 0.5, b_t, op0=MUL, op1=ADD)
            act.activation(tcS, S_sb, Tanh, scale=0.5)
            act.activation(so_lo, gp_Cl[:, :], Sigm)
            dve.tensor_mul(ring_lo[:, slot, :], so_lo, tcS[:, 0:128])
            act.activation(so_hi, gp_Ch[:, :], Sigm)
            dve.tensor_mul(ring_hi[:, slot, :], so_hi, tcS[:, 128:256])

            # transpose h -> hT next iteration (so this step's tail only
            # blocks the next step's stream, not the preadds)
            if tau < nc_T - 1:
                tp0 = tpool.tile([128, 128], bf16, tag="tp0", name="tp0")
                tp1 = tpool.tile([128, 128], bf16, tag="tp1", name="tp1")
                prev = (ring_lo[:, slot, :], ring_hi[:, slot, :])
            else:
                prev = None

            # ---- outputs ----
            # bwd h copied (gpsimd, off critical path) into a second ring
            # at reversed slot so both directions flush as one DMA per band
            # half per RING steps
            bslot = RING - 1 - slot
            pool.tensor_copy(ring_blo[:, bslot, :], ring_lo[:, slot, :])
            pool.tensor_copy(ring_bhi[:, bslot, :], ring_hi[:, slot, :])
            if slot == RING - 1 or tau == nc_T - 1:
                cnt = slot + 1
                t0 = tau - slot
                bt0 = nc_T - 1 - tau
                for b in range(4):
                    sdma.dma_start(
                        out=outd[t0:t0 + cnt, 0, b, :, 0:128]
                        .rearrange("t s n -> s t n"),
                        in_=ring_lo[32 * b:32 * b + 4, 0:cnt, :])
                    sdma.dma_start(
                        out=outd[t0:t0 + cnt, 0, b, :, 128:256]
                        .rearrange("t s n -> s t n"),
                        in_=ring_hi[32 * b:32 * b + 4, 0:cnt, :])
                    sdma.dma_start(
                        out=outd[bt0:bt0 + cnt, 1, b, :, 0:128]
                        .rearrange("t s n -> s t n"),
                        in_=ring_blo[32 * b + 4:32 * b + 8,
                                     RING - cnt:RING, :])
                    sdma.dma_start(
                        out=outd[bt0:bt0 + cnt, 1, b, :, 128:256]
                        .rearrange("t s n -> s t n"),
                        in_=ring_bhi[32 * b + 4:32 * b + 8,
                                     RING - cnt:RING, :])


def _prep_inputs(x, Wf, bf, Wi, bi, Wc, bc, Wo, bo, nc_T=T):
    """Host-side input preparation -> list of per-core in_maps."""
    import ml_dtypes
    # gate quarter order (f, g, i, o); f,i scaled 1/2 (sigmoid via tanh)
    W = np.stack([Wf, Wc, Wi, Wo], axis=1)          # (2048, 4, 1024)
    bv = np.stack([bf, bc, bi, bo], axis=0)         # (4, 1024)
    scale = np.array([0.5, 1.0, 0.5, 1.0], dtype=np.float32)
    W = W * scale[None, :, None]
    bv = bv * scale[:, None]
    # col = band*1024 + q*256 + n for H index band*256+n, quarter q
    W4 = np.ascontiguousarray(
        W.reshape(2048, 4, 4, 256).transpose(0, 2, 1, 3).reshape(2048, G4))
    b4 = np.ascontiguousarray(
        bv.reshape(4, 4, 256).transpose(1, 0, 2).reshape(G4))
    Wh = np.ascontiguousarray(W4[:H].astype(ml_dtypes.bfloat16))
    Wx = W4[H:]
    wxb = np.ascontiguousarray(
        np.concatenate([Wx, b4[None, :]], axis=0))  # (1025, 4096)
    # selection matrix rows (dirhalf, hi/lo part, band, seq); sums hi+lo.
    # 32 cols per band (cols 8..31 zero) so preadds write whole strips.
    # rows 64..67 select a per-band bias row carried in pfb rows 64:68;
    # rows 68:128 are explicit zeros (K padded to 128, a proven path)
    selm = np.zeros((128, 128), dtype=np.float32)
    for b_ in range(4):
        for sp in range(8):
            for part in range(2):
                selm[(sp // 4) * 32 + part * 16 + b_ * 4 + (sp % 4),
                     b_ * 32 + sp] = 1.0
            selm[64 + b_, b_ * 32 + sp] = 1.0
    selm = selm.astype(ml_dtypes.bfloat16)
    bz = np.zeros((64, 1024), dtype=np.float32)
    bz[0:4] = b4.reshape(4, 1024)
    bz = np.ascontiguousarray(bz.astype(ml_dtypes.bfloat16))
    id128 = np.ascontiguousarray(np.eye(128, dtype=np.float32)
                                 .astype(ml_dtypes.bfloat16))

    in_maps = []
    for c in range(NCORES):
        xc = x[BS * c:BS * c + BS, :nc_T, :].reshape(BS * nc_T, D)
        xTc = np.concatenate(
            [xc.T, np.ones((1, BS * nc_T), dtype=np.float32)], axis=0)
        in_maps.append({
            "xT": np.ascontiguousarray(xTc),
            "wxb": wxb,
            "wh": Wh,
            "sel": selm,
            "biasz": bz,
            "id128": id128,
        })
    return in_maps


def _assemble(results, nc_T=T):
    """results: list of dicts with 'out' (T, 2, 4, BS, 256) bf16 = h."""
    full = np.empty((B, nc_T, 2 * H), dtype=np.float32)
    for c in range(NCORES):
        o = np.asarray(results[c]["out"], dtype=np.float32)
        o2 = o.transpose(3, 0, 1, 2, 4).reshape(BS, nc_T, 2 * H)
        full[BS * c:BS * c + BS] = o2
    return full


def kernel(**inputs):
    global _BUILT
    from concourse.bass_utils import run_bass_kernel_spmd

    x = np.asarray(inputs["x"], dtype=np.float32)
    args = [np.asarray(inputs[k], dtype=np.float32)
            for k in ("Wf", "bf", "Wi", "bi", "Wc", "bc", "Wo", "bo")]
    in_maps = _prep_inputs(x, *args)
    if _BUILT is None:
        _BUILT = _build()
    res = run_bass_kernel_spmd(_BUILT, in_maps, core_ids=list(range(NCORES)))
    return _assemble(res.results)



# revision 2
# speedup vs baseline: 1.0699x; 1.0699x over previous
"""BiLSTM layer kernel for 8 Trainium2 NeuronCores — weight-stationary recurrence.

Problem: B=32, T=256, D=1024, H=1024 bidirectional LSTM, 8 cores.
Sharding: 4 batch rows x 2 directions = 8 sequences per core.

Key idea vs v2: in the recurrence, make Wh the STATIONARY operand and
h the MOVING operand. Gate outputs land as [128 gate-elems, 8 seqs]
PSUM tiles: per step 32 G-tiles x 8 k-tiles = 256 matmuls of N=8
columns each (2048 streamed columns/step vs 32768 in v2).  The h
produced by the elementwise phase is already in [H-elem, seq]
orientation, so the per-step PE transpose disappears.

Pre-gates (x@Wx+b) are computed once in phase 1 in row-major
orientation and kept entirely in SBUF (no DRAM roundtrip).  They are
injected into each step's PSUM bank via pred-as-stationary matmuls
with one-hot identity columns as the moving operand: row r = t*4+b of
strip w=t//32 is selected by identity column (t%32)*4+b.

Math: all four gates go through ONE activation type (tanh):
sigmoid(z) = (tanh(z/2)+1)/2 with the 1/2 baked into Wx/Wh/b for
f,i,o.  State carried as S=2c and H=2h (extra 1/2 baked into Wh):
  a = (tanh_f+1) * S ; b = (tanh_i+1) * tanh_g ; S' = 0.5a + b
  H = (tanh_o+1) * tanh(S'/2)
Host divides the output by 2 at assembly time.

Layouts (per core):
  xT   [1024 D, 1024 rows] bf16, row r = t*4+b
  wx   [1024 D, 4096 G]  bf16, G = g*1024 + j*128 + p, g in [f,i,g,o]
  wh   [1024 e, 4096 G]  bf16 (scaled by extra 0.5 for H=2h)
  bias [128, 4096] bf16 (broadcast over partitions)
  id   [128, 128] bf16 identity
  out  [128 p, 2 dir, 256 t, 32 (s*8+j)] bf16 = 2h
PSUM bank per step: [128, 256] f32, col = g*64 + s*8 + j.
"""

import numpy as np

B, T, D, H = 32, 256, 1024, 1024
NCORES = 8
BS = B // NCORES           # batch rows per core
KT = 8                     # k-tiles over contraction dims
G4 = 4 * H
RING = 16                  # output ring depth (steps per flush)

_BUILT = None


def _build(nc_T=T):
    import concourse.bass as bass
    import concourse.bacc as bacc
    import concourse.tile as tile
    from concourse import mybir

    bf16 = mybir.dt.bfloat16

    nc = bacc.Bacc("TRN2", target_bir_lowering=False)

    nrows = BS * nc_T
    xT = nc.dram_tensor("xT", [D, nrows], bf16, kind="ExternalInput")
    wx = nc.dram_tensor("wx", [D, G4], bf16, kind="ExternalInput")
    wh = nc.dram_tensor("wh", [H, G4], bf16, kind="ExternalInput")
    bias = nc.dram_tensor("bias", [128, G4], bf16, kind="ExternalInput")
    id128 = nc.dram_tensor("id128", [128, 128], bf16, kind="ExternalInput")
    outd = nc.dram_tensor("out", [128, 2, nc_T, 8 * BS], bf16,
                          kind="ExternalOutput")

    with tile.TileContext(nc) as tc:
        _emit(nc, tc, bass, mybir, nc_T, xT, wx, wh, bias, id128, outd)
    nc.finalize()
    return nc


def _emit(nc, tc, bass, mybir, nc_T, xT, wx, wh, bias, id128, outd):
    from contextlib import ExitStack

    f32 = mybir.dt.float32
    bf16 = mybir.dt.bfloat16
    Tanh = mybir.ActivationFunctionType.Tanh
    MUL = mybir.AluOpType.mult
    ADD = mybir.AluOpType.add

    nrows = BS * nc_T
    n_strips = nrows // 128          # row strips (8 for T=256)
    act = nc.scalar
    dve = nc.vector
    pe = nc.tensor
    sdma = nc.sync

    with ExitStack() as ctx:
        # ---------------- persistent tiles ----------------
        singles = ctx.enter_context(tc.tile_pool(name="singles", bufs=1))
        id_sb = singles.tile([128, 128], bf16)
        sdma.dma_start(out=id_sb, in_=id128[:, :])
        bias_sb = singles.tile([128, G4], bf16)

        whpool = ctx.enter_context(tc.tile_pool(name="whp", bufs=1))
        wh_sb = whpool.tile([128, KT, G4], bf16)

        predpool = ctx.enter_context(tc.tile_pool(name="predp", bufs=1))
        pred_sb = predpool.tile([128, n_strips, G4], bf16)

        # ---------------- phase 1: pred = x @ Wx + b ----------------
        with tc.tile_pool(name="p1x", bufs=1) as p1x, \
             tc.tile_pool(name="p1w", bufs=2) as p1w, \
             tc.tile_pool(name="p1ps", bufs=3, space="PSUM") as p1ps:
            xT_sb = p1x.tile([128, KT, nrows], bf16)

            wxn = [None] * 8

            def load_wxn(n):
                wxn[n] = p1w.tile([128, KT, 512], bf16, tag="wxn",
                                  name=f"wxn{n}")
                for k in range(KT):
                    sdma.dma_start(
                        out=wxn[n][:, k, :],
                        in_=wx[k * 128:(k + 1) * 128,
                               n * 512:(n + 1) * 512])

            # first wx strip interleaved with xT so the first matmuls
            # can start as early as possible
            wxn[0] = p1w.tile([128, KT, 512], bf16, tag="wxn", name="wxn0")
            for k in range(KT):
                sdma.dma_start(out=wxn[0][:, k, :],
                               in_=wx[k * 128:(k + 1) * 128, 0:512])
                sdma.dma_start(out=xT_sb[:, k, :],
                               in_=xT[k * 128:(k + 1) * 128, :])
            sdma.dma_start(out=bias_sb, in_=bias[:, :])
            load_wxn(1)
            for n in range(8):
                if n + 2 < 8:
                    load_wxn(n + 2)
                # wh is only needed in phase 2; spread its load out
                sdma.dma_start(out=wh_sb[:, n, :],
                               in_=wh[n * 128:(n + 1) * 128, :])
                for m in range(n_strips):
                    pt = p1ps.tile([128, 512], f32, name="pt")
                    for k in range(KT):
                        pe.matmul(
                            pt[:, :],
                            lhsT=xT_sb[:, k, m * 128:m * 128 + 128],
                            rhs=wxn[n][:, k, :],
                            start=(k == 0), stop=(k == KT - 1))
                    # evacuate + bias add in one DVE op
                    dve.scalar_tensor_tensor(
                        pred_sb[:, m, n * 512:(n + 1) * 512],
                        pt[:, :], 1.0,
                        bias_sb[:, n * 512:(n + 1) * 512],
                        op0=MUL, op1=ADD)

        # ---------------- phase 2: the recurrence ----------------
        state = ctx.enter_context(tc.tile_pool(name="state", bufs=1))
        ewpool = ctx.enter_context(tc.tile_pool(name="ew", bufs=2))
        ringpool = ctx.enter_context(tc.tile_pool(name="ring", bufs=2))
        gpool = ctx.enter_context(tc.tile_pool(name="gp", bufs=2,
                                               space="PSUM"))

        S_sb = state.tile([128, 64], f32, tag="S")   # 2*c, col = s*8+j
        dve.memset(S_sb, 0.0)

        h_prev = None   # AP of previous step's H (2h), col = s*8+j
        ring = None

        for tau in range(nc_T):
            slot = tau % RING
            if slot == 0:
                ring = ringpool.tile([128, RING, 64], bf16, tag="ring",
                                     name=f"ring{tau // RING}")

            ps = gpool.tile([128, 512], f32, tag="ps", name="ps")

            # ---- pre-gate injection (pred strips as stationary) ----
            tp = nc_T - 1 - tau
            wf, rf = tau // 32, tau % 32
            wb, rb = tp // 32, tp % 32
            first = True
            for g in range(4):
                for j in range(8):
                    tile_off = (g * 8 + j) * 128
                    c0 = g * 64 + j
                    pe.matmul(
                        ps[:, c0:c0 + 32:8],
                        lhsT=pred_sb[:, wf, tile_off:tile_off + 128],
                        rhs=id_sb[:, rf * 4:rf * 4 + 4],
                        start=first, stop=False,
                        skip_group_check=True)
                    first = False
                    pe.matmul(
                        ps[:, c0 + 32:g * 64 + 64:8],
                        lhsT=pred_sb[:, wb, tile_off:tile_off + 128],
                        rhs=id_sb[:, rb * 4:rb * 4 + 4],
                        start=False, stop=False,
                        skip_group_check=True)

            # ---- recurrent weight stream (o gate last) ----
            if tau > 0:
                for g in (0, 1, 2, 3):
                    for j in range(8):
                        tile_off = (g * 8 + j) * 128
                        c0 = g * 64 + j
                        for k in range(KT):
                            last = (g == 3 and j == 7 and k == KT - 1)
                            pe.matmul(
                                ps[:, c0:g * 64 + 64:8],
                                lhsT=wh_sb[:, k, tile_off:tile_off + 128],
                                rhs=h_prev[:, k:64:8],
                                start=False, stop=last,
                                skip_group_check=True)

            # ---- elementwise cell update ----
            tfig = ewpool.tile([128, 192], bf16, tag="tfig", name="tfig")
            to_t = ewpool.tile([128, 64], bf16, tag="to", name="to")
            a_t = ewpool.tile([128, 64], f32, tag="a", name="a_t")
            b_t = ewpool.tile([128, 64], f32, tag="b", name="b_t")
            tcS = ewpool.tile([128, 64], bf16, tag="tcS", name="tcS")

            act.activation(tfig, ps[:, 0:192], Tanh)
            act.activation(to_t, ps[:, 192:256], Tanh)
            dve.scalar_tensor_tensor(b_t, tfig[:, 64:128], 1.0,
                                     tfig[:, 128:192], op0=ADD, op1=MUL)
            dve.scalar_tensor_tensor(a_t, tfig[:, 0:64], 1.0, S_sb,
                                     op0=ADD, op1=MUL)
            dve.scalar_tensor_tensor(S_sb, a_t, 0.5, b_t, op0=MUL, op1=ADD)
            act.activation(tcS, S_sb, Tanh, scale=0.5)
            dve.scalar_tensor_tensor(ring[:, slot, :], to_t, 1.0, tcS,
                                     op0=ADD, op1=MUL)
            h_prev = ring[:, slot, :]

            # ---- output flush ----
            if slot == RING - 1 or tau == nc_T - 1:
                cnt = slot + 1
                t0 = tau - slot
                sdma.dma_start(
                    out=outd[:, 0, t0:t0 + cnt, :],
                    in_=ring[:, 0:cnt, 0:32])
                sdma.dma_start(
                    out=outd[:, 1, t0:t0 + cnt, :],
                    in_=ring[:, 0:cnt, 32:64])


def _prep_inputs(x, Wf, bf, Wi, bi, Wc, bc, Wo, bo, nc_T=T):
    """Host-side input preparation -> list of per-core in_maps."""
    import ml_dtypes
    # gate order f, i, g(=c), o; f,i,o scaled 1/2 (sigmoid via tanh)
    W = np.stack([Wf, Wi, Wc, Wo], axis=0)          # (4, 2048, 1024)
    bv = np.stack([bf, bi, bc, bo], axis=0)         # (4, 1024)
    scale = np.array([0.5, 0.5, 1.0, 0.5], dtype=np.float32)
    W = W * scale[:, None, None]
    bv = bv * scale[:, None]
    # G-col = g*1024 + h  (h = j*128 + p)
    Wpack = np.ascontiguousarray(W.transpose(1, 0, 2).reshape(2 * H, G4))
    Wh = np.ascontiguousarray((Wpack[:H] * 0.5).astype(ml_dtypes.bfloat16))
    Wx = np.ascontiguousarray(Wpack[H:].astype(ml_dtypes.bfloat16))
    b_pack = bv.reshape(G4)
    bias128 = np.ascontiguousarray(
        np.broadcast_to(b_pack, (128, G4)).astype(ml_dtypes.bfloat16))
    id128 = np.ascontiguousarray(np.eye(128, dtype=np.float32)
                                 .astype(ml_dtypes.bfloat16))

    in_maps = []
    for c in range(NCORES):
        # row r = t*4 + b
        xc = x[BS * c:BS * c + BS, :nc_T, :]         # (4, T, D)
        xr = xc.transpose(1, 0, 2).reshape(BS * nc_T, D)
        xTc = np.ascontiguousarray(xr.T.astype(ml_dtypes.bfloat16))
        in_maps.append({
            "xT": xTc,
            "wx": Wx,
            "wh": Wh,
            "bias": bias128,
            "id128": id128,
        })
    return in_maps


def _assemble(results, nc_T=T):
    """results: 'out' [128 p, 2 dir, T, 32 (s*8+j)] bf16 = 2h."""
    full = np.empty((B, nc_T, 2 * H), dtype=np.float32)
    for c in range(NCORES):
        o = np.asarray(results[c]["out"], dtype=np.float32) * 0.5
        # o[p, dir, t, s*8+j] -> (b=s, t, H=j*128+p)
        o = o.reshape(128, 2, nc_T, BS, 8)           # p, dir, t, b, j
        fwd = o[:, 0].transpose(2, 1, 3, 0).reshape(BS, nc_T, H)
        bwd = o[:, 1].transpose(2, 1, 3, 0).reshape(BS, nc_T, H)[:, ::-1, :]
        full[BS * c:BS * c + BS, :, :H] = fwd
        full[BS * c:BS * c + BS, :, H:] = bwd
    return full


def kernel(**inputs):
    global _BUILT
    from concourse.bass_utils import run_bass_kernel_spmd

    x = np.asarray(inputs["x"], dtype=np.float32)
    args = [np.asarray(inputs[k], dtype=np.float32)
            for k in ("Wf", "bf", "Wi", "bi", "Wc", "bc", "Wo", "bo")]
    in_maps = _prep_inputs(x, *args)
    if _BUILT is None:
        _BUILT = _build()
    res = run_bass_kernel_spmd(_BUILT, in_maps, core_ids=list(range(NCORES)))
    return _assemble(res.results)
